# revision 5
# baseline (speedup 1.0000x reference)
"""Trainium2 Bass kernel: LocalizedFiltering (2x causal conv k=2 + residual + RMSNorm)
over ragged packed sequences.

Strategy:
  - Split the 24576 packed tokens evenly across 8 NeuronCores (3072 tokens/core)
    with a 2-token halo; conv weights replicated.
  - Device computes conv1 -> (DRAM spill) -> conv2 + residual + RMSNorm on
    channels-on-partitions transposed layout, in float32r (full-rate PE matmul,
    RNE-11-bit mantissa) with fp32 PSUM accumulation.
  - Sequence-boundary rows (2 per sequence start) and the conv caches are
    recomputed exactly on host (a handful of matvecs).
"""
import numpy as np

import concourse.bass as bass
import concourse.tile as tile
from concourse import mybir
from concourse.bass_utils import run_bass_kernel_spmd
import bass_rust

EPS = 1e-6
P = 128
NCORES = 8
E = 2048
H = 1024
KE = E // P   # 16
KH = H // P   # 8
TT = 256      # tokens per device tile


# ---------------------------------------------------------------------------
# Wait legalization: the walrus build here allows at most one sync wait per
# instruction (two for EventSemaphore); hoist excess waits onto NoOps.
_WAIT_CAPS = {"Matmult": 0, "EventSemaphore": 2}


def _legalize_waits(nc):
    n_split = 0
    for fn in nc.m.functions:
        for bb in fn.blocks:
            insts = list(bb.instructions)
            out = []
            changed = False
            for ins in insts:
                si = ins.sync_info
                waits = list(si.on_wait) if si is not None else []
                cap = _WAIT_CAPS.get(ins.opcode, 1)
                if len(waits) > cap:
                    keep, hoist = waits[:cap], waits[cap:]
                    for w in hoist:
                        n_split += 1
                        nop = mybir.InstNoOp(
                            name=f"WSPLIT-{n_split}", ins=[], outs=[]
                        )
                        nop.engine = ins.engine
                        nop.sync_info = bass_rust.SyncInfo(on_wait=[w], on_update=[])
                        out.append(nop)
                    ins.sync_info = bass_rust.SyncInfo(
                        on_wait=keep, on_update=list(si.on_update)
                    )
                    changed = True
                out.append(ins)
            if changed:
                bb.instructions = out
    return n_split


# ---------------------------------------------------------------------------
# Host-side round-to-nearest-even to 11 mantissa bits (the f32r format the
# PE consumes; verified bit-exact against the DVE's fp32->f32r conversion).
def _rne11(a):
    u = np.ascontiguousarray(a, dtype=np.float32).view(np.uint32)
    drop = 12
    half = np.uint32(1 << (drop - 1))
    even = (~(u >> drop) & np.uint32(1)).astype(np.uint32)
    mask = np.uint32((0xFFFFFFFF >> drop) << drop)
    r = ((u + half - even) & mask).view(np.float32)
    return np.ascontiguousarray(r)


# ---------------------------------------------------------------------------
_CACHE = {}


def _build_device_program(TPC):
    NT = TPC // TT
    TTE = TT + 3   # X tile columns (2 halo + 1 even-pad for f32r)
    TO = TT + 2    # o1e columns per tile (even, required by f32r matmul)
    f32r = mybir.dt.float32r
    f32 = mybir.dt.float32

    nc = bass.Bass(target_bir_lowering=False)
    dxt = nc.dram_tensor("xt", [E, TPC + 4], f32r, kind="ExternalInput").ap()
    dw1 = nc.dram_tensor("w1t", [E, 2, H], f32r, kind="ExternalInput").ap()
    dw2 = nc.dram_tensor("w2t", [H, 2, E], f32r, kind="ExternalInput").ap()
    dxr = nc.dram_tensor("xres", [E, TPC], f32, kind="ExternalInput").ap()
    dgm = nc.dram_tensor("gam", [E], f32, kind="ExternalInput").ap()
    dyt = nc.dram_tensor("yt", [E, TPC], f32, kind="ExternalOutput").ap()

    xtr = dxt.rearrange("(k p) n -> p k n", p=P)      # [128, 16, TPC+2]
    xrr = dxr.rearrange("(k p) n -> p k n", p=P)      # [128, 16, TPC]
    ytr = dyt.rearrange("(k p) n -> p k n", p=P)      # [128, 16, TPC]
    w1r = dw1.rearrange("(k p) t m -> p k t m", p=P)  # [128, 16, 2, 1024]
    w2r = dw2.rearrange("(k p) t m -> p k t m", p=P)  # [128, 8, 2, 2048]
    gmr = dgm.rearrange("(k p) -> p k", p=P)          # [128, 16]

    with tile.TileContext(nc) as tc:
        with (
            tc.tile_pool(name="wpool", bufs=1) as wpool,
            tc.tile_pool(name="const", bufs=1) as constp,
            tc.tile_pool(name="act", bufs=2) as actp,
            tc.tile_pool(name="o1", bufs=2) as o1p,
            tc.tile_pool(name="sq", bufs=2) as sqp,
            tc.tile_pool(name="small", bufs=4) as smallp,
            tc.tile_pool(name="rb", bufs=2) as rbp,
            tc.tile_pool(name="mm", bufs=4, space="PSUM") as mmp,
            tc.tile_pool(name="vps", bufs=2, space="PSUM") as vpsp,
            tc.tile_pool(name="dram", bufs=2, space="DRAM") as dramp,
        ):
            ones_f = constp.tile([P, 1], f32)
            nc.vector.memset(ones_f, 1.0)
            ones_r = constp.tile([P, 1], f32r)
            nc.vector.tensor_copy(out=ones_r, in_=ones_f)
            epst = constp.tile([1, 1], f32)
            nc.vector.memset(epst, EPS)
            gamt = constp.tile([P, KE], f32)
            nc.sync.dma_start(out=gamt, in_=gmr)
            o1d = dramp.tile([NT, H, TO], f32r)

            # ---- pass A: conv1 (no bias; folded into xres) ----
            tw1 = wpool.tile([P, KE, 2, H], f32r, tag="w")
            nc.sync.dma_start(out=tw1, in_=w1r)
            for t in range(NT):
                xa = actp.tile([P, KE, TTE], f32r, tag="act")
                nc.sync.dma_start(out=xa, in_=xtr[:, :, t * TT : t * TT + TTE])
                o1sb = o1p.tile([P, KH, TO], f32r, tag="o1")
                for co in range(KH):
                    ps = mmp.tile([P, TO], f32, tag="mm")
                    n_mm = 0
                    for k in range(KE):
                        for tap in range(2):
                            nc.tensor.matmul(
                                ps,
                                tw1[:, k, tap, co * P : (co + 1) * P],
                                xa[:, k, tap : tap + TO],
                                start=(n_mm == 0),
                                stop=(n_mm == 2 * KE - 1),
                            )
                            n_mm += 1
                    nc.vector.tensor_copy(out=o1sb[:, co, :], in_=ps)
                nc.sync.dma_start(
                    out=o1d[t].rearrange("(k p) n -> p k n", p=P), in_=o1sb
                )

            # ---- pass B: conv2 + residual (+b2eff via xres) + RMSNorm ----
            tw2 = wpool.tile([P, KH, 2, E], f32r, tag="w")
            nc.sync.dma_start(out=tw2, in_=w2r)
            for t in range(NT):
                o1t = o1p.tile([P, KH, TO], f32r, tag="o1")
                nc.sync.dma_start(
                    out=o1t, in_=o1d[t].rearrange("(k p) n -> p k n", p=P)
                )
                rt = actp.tile([P, KE, TT], f32, tag="act")
                nc.sync.dma_start(out=rt, in_=xrr[:, :, t * TT : (t + 1) * TT])
                vps = vpsp.tile([1, TT], f32, tag="v")
                for co in range(KE):
                    ps2 = mmp.tile([P, TT], f32, tag="mm")
                    n_mm = 0
                    for k in range(KH):
                        for tap in range(2):
                            nc.tensor.matmul(
                                ps2,
                                tw2[:, k, tap, co * P : (co + 1) * P],
                                o1t[:, k, tap : tap + TT],
                                start=(n_mm == 0),
                                stop=(n_mm == 2 * KH - 1),
                            )
                            n_mm += 1
                    nc.vector.tensor_add(
                        out=rt[:, co, :], in0=rt[:, co, :], in1=ps2
                    )
                    sq = sqp.tile([P, TT], f32r, tag="sq")
                    nc.vector.tensor_mul(
                        out=sq, in0=rt[:, co, :], in1=rt[:, co, :]
                    )
                    nc.tensor.matmul(
                        vps,
                        ones_r,
                        sq,
                        start=(co == 0),
                        stop=(co == KE - 1),
                        skip_group_check=True,
                    )
                vsb = smallp.tile([1, TT], f32)
                nc.scalar.activation(
                    out=vsb,
                    in_=vps,
                    func=mybir.ActivationFunctionType.Sqrt,
                    bias=epst,
                    scale=1.0 / E,
                )
                rec = smallp.tile([1, TT], f32)
                nc.vector.reciprocal(out=rec, in_=vsb)
                dvrec = dramp.tile([1, TT], f32, tag="vrec")
                nc.sync.dma_start(out=dvrec, in_=rec)
                rb = rbp.tile([P, TT], f32)
                nc.sync.dma_start(out=rb, in_=dvrec.to_broadcast([P, TT]))
                for co in range(KE):
                    nc.vector.tensor_mul(
                        out=rt[:, co, :], in0=rt[:, co, :], in1=rb
                    )
                    nc.vector.tensor_scalar_mul(
                        out=rt[:, co, :], in0=rt[:, co, :], scalar1=gamt[:, co : co + 1]
                    )
                nc.sync.dma_start(out=ytr[:, :, t * TT : (t + 1) * TT], in_=rt)

    _legalize_waits(nc)
    return nc


def _get_program(TPC):
    if TPC not in _CACHE:
        _CACHE[TPC] = _build_device_program(TPC)
    return _CACHE[TPC]


_LAST_IN_MAPS = None


def _device_run(nc, in_maps, **kw):
    return run_bass_kernel_spmd(nc, in_maps, core_ids=list(range(NCORES)), **kw)


# ---------------------------------------------------------------------------
def kernel(inputs, lf1_cache, lf2_cache, w1, b1, w2, b2, gamma,
           seq_start_loc, seq_lens, max_len):
    global _LAST_IN_MAPS
    inputs = np.asarray(inputs, np.float32)
    lf1_cache = np.asarray(lf1_cache, np.float32)
    lf2_cache = np.asarray(lf2_cache, np.float32)
    w1 = np.asarray(w1, np.float32)
    b1 = np.asarray(b1, np.float32)
    w2 = np.asarray(w2, np.float32)
    b2 = np.asarray(b2, np.float32)
    gamma = np.asarray(gamma, np.float32)
    seq_start_loc = np.asarray(seq_start_loc, np.int64)
    seq_lens = np.asarray(seq_lens, np.int64)
    max_len = int(np.asarray(max_len))

    T, E_ = inputs.shape
    B = seq_lens.shape[0]
    assert E_ == E and T % NCORES == 0
    TPC = T // NCORES
    assert TPC % TT == 0

    nc = _get_program(TPC)

    # host prep
    b2eff = b2 + (w2[:, :, 0] + w2[:, :, 1]) @ b1            # [E]
    w1t = _rne11(np.transpose(w1, (1, 2, 0)))                # [E, 2, H]
    w2t = _rne11(np.transpose(w2, (1, 2, 0)))                # [H, 2, E]
    XE = np.concatenate(
        [np.zeros((2, E), np.float32), inputs, np.zeros((2, E), np.float32)],
        axis=0,
    )

    in_maps = []
    for c in range(NCORES):
        xe_c = XE[c * TPC : c * TPC + TPC + 4]               # [TPC+4, E]
        xt = _rne11(xe_c.T)                                  # [E, TPC+4]
        xres = np.ascontiguousarray(inputs[c * TPC : (c + 1) * TPC].T) \
            + b2eff[:, None]                                 # [E, TPC]
        in_maps.append({
            "xt": xt,
            "w1t": w1t,
            "w2t": w2t,
            "xres": np.ascontiguousarray(xres, np.float32),
            "gam": gamma,
        })
    _LAST_IN_MAPS = in_maps

    res = _device_run(nc, in_maps)
    y = np.concatenate(
        [np.ascontiguousarray(res.results[c]["yt"]).T for c in range(NCORES)],
        axis=0,
    )  # [T, E]

    # ---- host fixups: rows s0, s0+1 of each sequence (exact) ----
    W10, W11 = w1[:, :, 0], w1[:, :, 1]
    W20, W21 = w2[:, :, 0], w2[:, :, 1]

    def _norm(out_row):
        var = np.mean(out_row.astype(np.float32) ** 2)
        return gamma * (out_row * (1.0 / np.sqrt(var + EPS)))

    for i in range(B):
        L = int(seq_lens[i])
        s0 = int(seq_start_loc[i])
        if L <= 0:
            continue
        p0 = max_len - L
        x0 = inputs[s0]
        if p0 == 0:
            xm1 = lf1_cache[i, :, 0, 0]
            o1_m1 = lf2_cache[i, :, 0, 0]
        elif p0 == 1:
            xm1 = np.zeros(E, np.float32)
            o1_m1 = W10 @ lf1_cache[i, :, 0, 0] + b1
        else:
            xm1 = np.zeros(E, np.float32)
            o1_m1 = b1
        o1_0 = W10 @ xm1 + W11 @ x0 + b1
        o2_0 = W20 @ o1_m1 + W21 @ o1_0 + b2
        y[s0] = _norm(o2_0 + x0)
        if L >= 2:
            x1 = inputs[s0 + 1]
            o1_1 = W10 @ x0 + W11 @ x1 + b1
            o2_1 = W20 @ o1_0 + W21 @ o1_1 + b2
            y[s0 + 1] = _norm(o2_1 + x1)

    # ---- caches ----
    lf1 = np.zeros((B, E, 1, 1), np.float32)
    lf2 = np.zeros((B, H, 1, 1), np.float32)
    for i in range(B):
        L = int(seq_lens[i])
        s0 = int(seq_start_loc[i])
        if L <= 0:
            continue
        p0 = max_len - L
        xlast = inputs[s0 + L - 1]
        if L >= 2:
            xprev = inputs[s0 + L - 2]
        elif p0 == 0:
            xprev = lf1_cache[i, :, 0, 0]
        else:
            xprev = np.zeros(E, np.float32)
        lf1[i, :, 0, 0] = xlast
        lf2[i, :, 0, 0] = W10 @ xprev + W11 @ xlast + b1

    return y, lf1, lf2


# revision 7
# speedup vs baseline: 1.0092x; 1.0092x over previous
"""Trainium2 Bass kernel: LocalizedFiltering (2x causal conv k=2 + residual + RMSNorm)
over ragged packed sequences.

Strategy:
  - Split the 24576 packed tokens evenly across 8 NeuronCores (3072 tokens/core)
    with a 2-token halo; conv weights replicated.
  - Device computes conv1 -> (DRAM spill) -> conv2 + residual + RMSNorm on
    channels-on-partitions transposed layout, in float32r (full-rate PE matmul,
    RNE-11-bit mantissa) with fp32 PSUM accumulation.
  - Sequence-boundary rows (2 per sequence start) and the conv caches are
    recomputed exactly on host (a handful of matvecs).
"""
import numpy as np

import concourse.bass as bass
import concourse.tile as tile
from concourse import mybir
from concourse.bass_utils import run_bass_kernel_spmd
import bass_rust

EPS = 1e-6
P = 128
NCORES = 8
E = 2048
H = 1024
KE = E // P   # 16
KH = H // P   # 8
TT = 256      # tokens per device tile


# ---------------------------------------------------------------------------
# Wait legalization: the walrus build here allows at most one sync wait per
# instruction (two for EventSemaphore); hoist excess waits onto NoOps.
_WAIT_CAPS = {"Matmult": 0, "EventSemaphore": 2}


def _legalize_waits(nc):
    n_split = 0
    for fn in nc.m.functions:
        for bb in fn.blocks:
            insts = list(bb.instructions)
            out = []
            changed = False
            for ins in insts:
                si = ins.sync_info
                waits = list(si.on_wait) if si is not None else []
                cap = _WAIT_CAPS.get(ins.opcode, 1)
                if len(waits) > cap:
                    keep, hoist = waits[:cap], waits[cap:]
                    for w in hoist:
                        n_split += 1
                        nop = mybir.InstNoOp(
                            name=f"WSPLIT-{n_split}", ins=[], outs=[]
                        )
                        nop.engine = ins.engine
                        nop.sync_info = bass_rust.SyncInfo(on_wait=[w], on_update=[])
                        out.append(nop)
                    ins.sync_info = bass_rust.SyncInfo(
                        on_wait=keep, on_update=list(si.on_update)
                    )
                    changed = True
                out.append(ins)
            if changed:
                bb.instructions = out
    return n_split


# ---------------------------------------------------------------------------
# Host-side round-to-nearest-even to 11 mantissa bits (the f32r format the
# PE consumes; verified bit-exact against the DVE's fp32->f32r conversion).
def _rne11(a):
    u = np.ascontiguousarray(a, dtype=np.float32).view(np.uint32)
    drop = 12
    half = np.uint32(1 << (drop - 1))
    even = (~(u >> drop) & np.uint32(1)).astype(np.uint32)
    mask = np.uint32((0xFFFFFFFF >> drop) << drop)
    r = ((u + half - even) & mask).view(np.float32)
    return np.ascontiguousarray(r)


# ---------------------------------------------------------------------------
_CACHE = {}


def _build_device_program(TPC):
    NT = TPC // TT
    TTE = TT + 3   # X tile columns (2 halo + 1 even-pad for f32r)
    TO = TT + 2    # o1e columns per tile (even, required by f32r matmul)
    f32r = mybir.dt.float32r
    f32 = mybir.dt.float32

    nc = bass.Bass(target_bir_lowering=False)
    dxt = nc.dram_tensor("xt", [E, TPC + 4], f32r, kind="ExternalInput").ap()
    dw1 = nc.dram_tensor("w1t", [E, 2, H], f32r, kind="ExternalInput").ap()
    dw2 = nc.dram_tensor("w2t", [H, 2, E], f32r, kind="ExternalInput").ap()
    dxr = nc.dram_tensor("xres", [E, TPC], f32, kind="ExternalInput").ap()
    dgm = nc.dram_tensor("gam", [E], f32, kind="ExternalInput").ap()
    dyt = nc.dram_tensor("yt", [E, TPC], f32, kind="ExternalOutput").ap()

    xtr = dxt.rearrange("(k p) n -> p k n", p=P)      # [128, 16, TPC+2]
    xrr = dxr.rearrange("(k p) n -> p k n", p=P)      # [128, 16, TPC]
    ytr = dyt.rearrange("(k p) n -> p k n", p=P)      # [128, 16, TPC]
    w1r = dw1.rearrange("(k p) t m -> p k t m", p=P)  # [128, 16, 2, 1024]
    w2r = dw2.rearrange("(k p) t m -> p k t m", p=P)  # [128, 8, 2, 2048]
    gmr = dgm.rearrange("(k p) -> p k", p=P)          # [128, 16]

    with tile.TileContext(nc) as tc:
        with (
            tc.tile_pool(name="wpool", bufs=1) as wpool,
            tc.tile_pool(name="const", bufs=1) as constp,
            tc.tile_pool(name="act", bufs=2) as actp,
            tc.tile_pool(name="o1", bufs=2) as o1p,
            tc.tile_pool(name="sq", bufs=3) as sqp,
            tc.tile_pool(name="small", bufs=4) as smallp,
            tc.tile_pool(name="rb", bufs=2) as rbp,
            tc.tile_pool(name="mm", bufs=4, space="PSUM") as mmp,
            tc.tile_pool(name="vps", bufs=2, space="PSUM") as vpsp,
            tc.tile_pool(name="dram", bufs=2, space="DRAM") as dramp,
        ):
            ones_f = constp.tile([P, 1], f32)
            nc.vector.memset(ones_f, 1.0)
            ones_r = constp.tile([P, 1], f32r)
            nc.vector.tensor_copy(out=ones_r, in_=ones_f)
            epst = constp.tile([1, 1], f32)
            nc.vector.memset(epst, EPS)
            gamt = constp.tile([P, KE], f32)
            nc.sync.dma_start(out=gamt, in_=gmr)
            o1d = dramp.tile([NT, H, TO], f32r)

            # ---- pass A: conv1 (no bias; folded into xres) ----
            tw1 = wpool.tile([P, KE, 2, H], f32r, tag="w")
            for k in range(KE):
                nc.sync.dma_start(out=tw1[:, k, :, :], in_=w1r[:, k, :, :])
            for t in range(NT):
                xa = actp.tile([P, KE, TTE], f32r, tag="act")
                nc.sync.dma_start(out=xa, in_=xtr[:, :, t * TT : t * TT + TTE])
                o1sb = o1p.tile([P, KH, TO], f32r, tag="o1")
                for co in range(KH):
                    ps = mmp.tile([P, TO], f32, tag="mm")
                    n_mm = 0
                    for k in range(KE):
                        for tap in range(2):
                            nc.tensor.matmul(
                                ps,
                                tw1[:, k, tap, co * P : (co + 1) * P],
                                xa[:, k, tap : tap + TO],
                                start=(n_mm == 0),
                                stop=(n_mm == 2 * KE - 1),
                            )
                            n_mm += 1
                    nc.vector.tensor_copy(out=o1sb[:, co, :], in_=ps)
                nc.sync.dma_start(
                    out=o1d[t].rearrange("(k p) n -> p k n", p=P), in_=o1sb
                )

            # ---- pass B: conv2 + residual (+b2eff via xres) + RMSNorm ----
            tw2 = wpool.tile([P, KH, 2, E], f32r, tag="w")
            for k in range(KH):
                nc.sync.dma_start(out=tw2[:, k, :, :], in_=w2r[:, k, :, :])
            for t in range(NT):
                o1t = o1p.tile([P, KH, TO], f32r, tag="o1")
                nc.sync.dma_start(
                    out=o1t, in_=o1d[t].rearrange("(k p) n -> p k n", p=P)
                )
                rt = actp.tile([P, KE, TT], f32, tag="act")
                nc.sync.dma_start(out=rt, in_=xrr[:, :, t * TT : (t + 1) * TT])
                vps = vpsp.tile([1, TT], f32, tag="v")
                sq_tiles = [None] * KE
                for co in range(KE):
                    ps2 = mmp.tile([P, TT], f32, tag="mm")
                    n_mm = 0
                    for k in range(KH):
                        for tap in range(2):
                            nc.tensor.matmul(
                                ps2,
                                tw2[:, k, tap, co * P : (co + 1) * P],
                                o1t[:, k, tap : tap + TT],
                                start=(n_mm == 0),
                                stop=(n_mm == 2 * KH - 1),
                            )
                            n_mm += 1
                    nc.vector.tensor_add(
                        out=rt[:, co, :], in0=rt[:, co, :], in1=ps2
                    )
                    sq = sqp.tile([P, TT], f32r, tag="sq")
                    nc.vector.tensor_mul(
                        out=sq, in0=rt[:, co, :], in1=rt[:, co, :]
                    )
                    sq_tiles[co] = sq
                    # issue the variance matmul one chunk late: its DVE input
                    # is then already materialized when PE reaches it
                    if co >= 1:
                        nc.tensor.matmul(
                            vps, ones_r, sq_tiles[co - 1],
                            start=(co == 1), stop=False,
                            skip_group_check=True,
                        )
                nc.tensor.matmul(
                    vps, ones_r, sq_tiles[KE - 1],
                    start=False, stop=True, skip_group_check=True,
                )
                vsb = smallp.tile([1, TT], f32)
                nc.scalar.activation(
                    out=vsb,
                    in_=vps,
                    func=mybir.ActivationFunctionType.Sqrt,
                    bias=epst,
                    scale=1.0 / E,
                )
                rec = smallp.tile([1, TT], f32)
                nc.vector.reciprocal(out=rec, in_=vsb)
                dvrec = dramp.tile([1, TT], f32, tag="vrec")
                nc.sync.dma_start(out=dvrec, in_=rec)
                rb = rbp.tile([P, TT], f32)
                nc.sync.dma_start(out=rb, in_=dvrec.to_broadcast([P, TT]))
                for co in range(KE):
                    nc.vector.tensor_mul(
                        out=rt[:, co, :], in0=rt[:, co, :], in1=rb
                    )
                    nc.vector.tensor_scalar_mul(
                        out=rt[:, co, :], in0=rt[:, co, :], scalar1=gamt[:, co : co + 1]
                    )
                nc.sync.dma_start(out=ytr[:, :, t * TT : (t + 1) * TT], in_=rt)

    _legalize_waits(nc)
    return nc


def _get_program(TPC):
    if TPC not in _CACHE:
        _CACHE[TPC] = _build_device_program(TPC)
    return _CACHE[TPC]


_LAST_IN_MAPS = None


def _device_run(nc, in_maps, **kw):
    return run_bass_kernel_spmd(nc, in_maps, core_ids=list(range(NCORES)), **kw)


# ---------------------------------------------------------------------------
def kernel(inputs, lf1_cache, lf2_cache, w1, b1, w2, b2, gamma,
           seq_start_loc, seq_lens, max_len):
    global _LAST_IN_MAPS
    inputs = np.asarray(inputs, np.float32)
    lf1_cache = np.asarray(lf1_cache, np.float32)
    lf2_cache = np.asarray(lf2_cache, np.float32)
    w1 = np.asarray(w1, np.float32)
    b1 = np.asarray(b1, np.float32)
    w2 = np.asarray(w2, np.float32)
    b2 = np.asarray(b2, np.float32)
    gamma = np.asarray(gamma, np.float32)
    seq_start_loc = np.asarray(seq_start_loc, np.int64)
    seq_lens = np.asarray(seq_lens, np.int64)
    max_len = int(np.asarray(max_len))

    T, E_ = inputs.shape
    B = seq_lens.shape[0]
    assert E_ == E and T % NCORES == 0
    TPC = T // NCORES
    assert TPC % TT == 0

    nc = _get_program(TPC)

    # host prep
    b2eff = b2 + (w2[:, :, 0] + w2[:, :, 1]) @ b1            # [E]
    w1t = _rne11(np.transpose(w1, (1, 2, 0)))                # [E, 2, H]
    w2t = _rne11(np.transpose(w2, (1, 2, 0)))                # [H, 2, E]
    XE = np.concatenate(
        [np.zeros((2, E), np.float32), inputs, np.zeros((2, E), np.float32)],
        axis=0,
    )

    in_maps = []
    for c in range(NCORES):
        xe_c = XE[c * TPC : c * TPC + TPC + 4]               # [TPC+4, E]
        xt = _rne11(xe_c.T)                                  # [E, TPC+4]
        xres = np.ascontiguousarray(inputs[c * TPC : (c + 1) * TPC].T) \
            + b2eff[:, None]                                 # [E, TPC]
        in_maps.append({
            "xt": xt,
            "w1t": w1t,
            "w2t": w2t,
            "xres": np.ascontiguousarray(xres, np.float32),
            "gam": gamma,
        })
    _LAST_IN_MAPS = in_maps

    res = _device_run(nc, in_maps)
    y = np.concatenate(
        [np.ascontiguousarray(res.results[c]["yt"]).T for c in range(NCORES)],
        axis=0,
    )  # [T, E]

    # ---- host fixups: rows s0, s0+1 of each sequence (exact) ----
    W10, W11 = w1[:, :, 0], w1[:, :, 1]
    W20, W21 = w2[:, :, 0], w2[:, :, 1]

    def _norm(out_row):
        var = np.mean(out_row.astype(np.float32) ** 2)
        return gamma * (out_row * (1.0 / np.sqrt(var + EPS)))

    for i in range(B):
        L = int(seq_lens[i])
        s0 = int(seq_start_loc[i])
        if L <= 0:
            continue
        p0 = max_len - L
        x0 = inputs[s0]
        if p0 == 0:
            xm1 = lf1_cache[i, :, 0, 0]
            o1_m1 = lf2_cache[i, :, 0, 0]
        elif p0 == 1:
            xm1 = np.zeros(E, np.float32)
            o1_m1 = W10 @ lf1_cache[i, :, 0, 0] + b1
        else:
            xm1 = np.zeros(E, np.float32)
            o1_m1 = b1
        o1_0 = W10 @ xm1 + W11 @ x0 + b1
        o2_0 = W20 @ o1_m1 + W21 @ o1_0 + b2
        y[s0] = _norm(o2_0 + x0)
        if L >= 2:
            x1 = inputs[s0 + 1]
            o1_1 = W10 @ x0 + W11 @ x1 + b1
            o2_1 = W20 @ o1_0 + W21 @ o1_1 + b2
            y[s0 + 1] = _norm(o2_1 + x1)

    # ---- caches ----
    lf1 = np.zeros((B, E, 1, 1), np.float32)
    lf2 = np.zeros((B, H, 1, 1), np.float32)
    for i in range(B):
        L = int(seq_lens[i])
        s0 = int(seq_start_loc[i])
        if L <= 0:
            continue
        p0 = max_len - L
        xlast = inputs[s0 + L - 1]
        if L >= 2:
            xprev = inputs[s0 + L - 2]
        elif p0 == 0:
            xprev = lf1_cache[i, :, 0, 0]
        else:
            xprev = np.zeros(E, np.float32)
        lf1[i, :, 0, 0] = xlast
        lf2[i, :, 0, 0] = W10 @ xprev + W11 @ xlast + b1

    return y, lf1, lf2


# revision 8
# speedup vs baseline: 1.0261x; 1.0168x over previous
"""Trainium2 Bass kernel: LocalizedFiltering (2x causal conv k=2 + residual + RMSNorm)
over ragged packed sequences.

Strategy:
  - Split the 24576 packed tokens evenly across 8 NeuronCores (3072 tokens/core)
    with a 2-token halo; conv weights replicated.
  - Device computes conv1 -> (DRAM spill) -> conv2 + residual + RMSNorm on
    channels-on-partitions transposed layout, in float32r (full-rate PE matmul,
    RNE-11-bit mantissa) with fp32 PSUM accumulation.
  - Sequence-boundary rows (2 per sequence start) and the conv caches are
    recomputed exactly on host (a handful of matvecs).
"""
import numpy as np

import concourse.bass as bass
import concourse.tile as tile
from concourse import mybir
from concourse.bass_utils import run_bass_kernel_spmd
import bass_rust

EPS = 1e-6
P = 128
NCORES = 8
E = 2048
H = 1024
KE = E // P   # 16
KH = H // P   # 8
TT = 256      # tokens per device tile


# ---------------------------------------------------------------------------
# Wait legalization: the walrus build here allows at most one sync wait per
# instruction (two for EventSemaphore); hoist excess waits onto NoOps.
_WAIT_CAPS = {"Matmult": 0, "EventSemaphore": 2}


def _legalize_waits(nc):
    n_split = 0
    for fn in nc.m.functions:
        for bb in fn.blocks:
            insts = list(bb.instructions)
            out = []
            changed = False
            for ins in insts:
                si = ins.sync_info
                waits = list(si.on_wait) if si is not None else []
                cap = _WAIT_CAPS.get(ins.opcode, 1)
                if len(waits) > cap:
                    keep, hoist = waits[:cap], waits[cap:]
                    for w in hoist:
                        n_split += 1
                        nop = mybir.InstNoOp(
                            name=f"WSPLIT-{n_split}", ins=[], outs=[]
                        )
                        nop.engine = ins.engine
                        nop.sync_info = bass_rust.SyncInfo(on_wait=[w], on_update=[])
                        out.append(nop)
                    ins.sync_info = bass_rust.SyncInfo(
                        on_wait=keep, on_update=list(si.on_update)
                    )
                    changed = True
                out.append(ins)
            if changed:
                bb.instructions = out
    return n_split


# ---------------------------------------------------------------------------
# Host-side round-to-nearest-even to 11 mantissa bits (the f32r format the
# PE consumes; verified bit-exact against the DVE's fp32->f32r conversion).
def _rne11(a):
    u = np.ascontiguousarray(a, dtype=np.float32).view(np.uint32)
    drop = 12
    half = np.uint32(1 << (drop - 1))
    even = (~(u >> drop) & np.uint32(1)).astype(np.uint32)
    mask = np.uint32((0xFFFFFFFF >> drop) << drop)
    r = ((u + half - even) & mask).view(np.float32)
    return np.ascontiguousarray(r)


# ---------------------------------------------------------------------------
_CACHE = {}


def _build_device_program(TPC):
    NT = TPC // TT
    TTE = TT + 3   # X tile columns (2 halo + 1 even-pad for f32r)
    TO = TT + 2    # o1e columns per tile (even, required by f32r matmul)
    f32r = mybir.dt.float32r
    f32 = mybir.dt.float32

    nc = bass.Bass(target_bir_lowering=False)
    dxt = nc.dram_tensor("xt", [E, TPC + 4], f32r, kind="ExternalInput").ap()
    dw1 = nc.dram_tensor("w1t", [E, 2, H], f32r, kind="ExternalInput").ap()
    dw2 = nc.dram_tensor("w2t", [H, 2, E], f32r, kind="ExternalInput").ap()
    dxr = nc.dram_tensor("xres", [E, TPC], f32, kind="ExternalInput").ap()
    dgm = nc.dram_tensor("gam", [E], f32, kind="ExternalInput").ap()
    dyt = nc.dram_tensor("yt", [E, TPC], f32, kind="ExternalOutput").ap()

    xtr = dxt.rearrange("(k p) n -> p k n", p=P)      # [128, 16, TPC+2]
    xrr = dxr.rearrange("(k p) n -> p k n", p=P)      # [128, 16, TPC]
    ytr = dyt.rearrange("(k p) n -> p k n", p=P)      # [128, 16, TPC]
    w1r = dw1.rearrange("(k p) t m -> p k t m", p=P)  # [128, 16, 2, 1024]
    w2r = dw2.rearrange("(k p) t m -> p k t m", p=P)  # [128, 8, 2, 2048]
    gmr = dgm.rearrange("(k p) -> p k", p=P)          # [128, 16]

    with tile.TileContext(nc) as tc:
        with (
            tc.tile_pool(name="wpool", bufs=1) as wpool,
            tc.tile_pool(name="const", bufs=1) as constp,
            tc.tile_pool(name="act", bufs=2) as actp,
            tc.tile_pool(name="o1", bufs=2) as o1p,
            tc.tile_pool(name="sq", bufs=3) as sqp,
            tc.tile_pool(name="small", bufs=4) as smallp,
            tc.tile_pool(name="rb", bufs=2) as rbp,
            tc.tile_pool(name="mm", bufs=6, space="PSUM") as mmp,
            tc.tile_pool(name="vps", bufs=2, space="PSUM") as vpsp,
            tc.tile_pool(name="dram", bufs=2, space="DRAM") as dramp,
        ):
            ones_f = constp.tile([P, 1], f32)
            nc.vector.memset(ones_f, 1.0)
            ones_r = constp.tile([P, 1], f32r)
            nc.vector.tensor_copy(out=ones_r, in_=ones_f)
            epst = constp.tile([1, 1], f32)
            nc.vector.memset(epst, EPS)
            gamt = constp.tile([P, KE], f32)
            nc.scalar.dma_start(out=gamt, in_=gmr)
            o1d = dramp.tile([NT, H, TO], f32r)

            # ---- pass A: conv1 (no bias; folded into xres) ----
            tw1 = wpool.tile([P, KE, 2, H], f32r, tag="w")
            for k in range(KE):
                nc.sync.dma_start(out=tw1[:, k, :, :], in_=w1r[:, k, :, :])
            for t in range(NT):
                xa = actp.tile([P, KE, TTE], f32r, tag="act")
                nc.scalar.dma_start(out=xa, in_=xtr[:, :, t * TT : t * TT + TTE])
                o1sb = o1p.tile([P, KH, TO], f32r, tag="o1")
                for co in range(KH):
                    ps = mmp.tile([P, TO], f32, tag="mm")
                    n_mm = 0
                    for k in range(KE):
                        for tap in range(2):
                            nc.tensor.matmul(
                                ps,
                                tw1[:, k, tap, co * P : (co + 1) * P],
                                xa[:, k, tap : tap + TO],
                                start=(n_mm == 0),
                                stop=(n_mm == 2 * KE - 1),
                            )
                            n_mm += 1
                    nc.vector.tensor_copy(out=o1sb[:, co, :], in_=ps)
                nc.gpsimd.dma_start(
                    out=o1d[t].rearrange("(k p) n -> p k n", p=P), in_=o1sb
                )

            # ---- pass B: conv2 + residual (+b2eff via xres) + RMSNorm ----
            tw2 = wpool.tile([P, KH, 2, E], f32r, tag="w")
            for k in range(KH):
                nc.sync.dma_start(out=tw2[:, k, :, :], in_=w2r[:, k, :, :])
            for t in range(NT):
                o1t = o1p.tile([P, KH, TO], f32r, tag="o1")
                nc.scalar.dma_start(
                    out=o1t, in_=o1d[t].rearrange("(k p) n -> p k n", p=P)
                )
                rt = actp.tile([P, KE, TT], f32, tag="act")
                nc.scalar.dma_start(out=rt, in_=xrr[:, :, t * TT : (t + 1) * TT])
                vps = vpsp.tile([1, TT], f32, tag="v")
                sq_tiles = [None] * KE
                for co in range(KE):
                    ps2 = mmp.tile([P, TT], f32, tag="mm")
                    n_mm = 0
                    for k in range(KH):
                        for tap in range(2):
                            nc.tensor.matmul(
                                ps2,
                                tw2[:, k, tap, co * P : (co + 1) * P],
                                o1t[:, k, tap : tap + TT],
                                start=(n_mm == 0),
                                stop=(n_mm == 2 * KH - 1),
                            )
                            n_mm += 1
                    nc.vector.tensor_add(
                        out=rt[:, co, :], in0=rt[:, co, :], in1=ps2
                    )
                    sq = sqp.tile([P, TT], f32r, tag="sq")
                    nc.vector.tensor_mul(
                        out=sq, in0=rt[:, co, :], in1=rt[:, co, :]
                    )
                    sq_tiles[co] = sq
                    # issue the variance matmul one chunk late: its DVE input
                    # is then already materialized when PE reaches it
                    if co >= 1:
                        nc.tensor.matmul(
                            vps, ones_r, sq_tiles[co - 1],
                            start=(co == 1), stop=False,
                            skip_group_check=True,
                        )
                nc.tensor.matmul(
                    vps, ones_r, sq_tiles[KE - 1],
                    start=False, stop=True, skip_group_check=True,
                )
                vsb = smallp.tile([1, TT], f32)
                nc.scalar.activation(
                    out=vsb,
                    in_=vps,
                    func=mybir.ActivationFunctionType.Sqrt,
                    bias=epst,
                    scale=1.0 / E,
                )
                rec = smallp.tile([1, TT], f32)
                nc.vector.reciprocal(out=rec, in_=vsb)
                dvrec = dramp.tile([1, TT], f32, tag="vrec")
                nc.gpsimd.dma_start(out=dvrec, in_=rec)
                rb = rbp.tile([P, TT], f32)
                nc.gpsimd.dma_start(out=rb, in_=dvrec.to_broadcast([P, TT]))
                for co in range(KE):
                    nc.vector.tensor_mul(
                        out=rt[:, co, :], in0=rt[:, co, :], in1=rb
                    )
                    nc.vector.tensor_scalar_mul(
                        out=rt[:, co, :], in0=rt[:, co, :], scalar1=gamt[:, co : co + 1]
                    )
                nc.gpsimd.dma_start(out=ytr[:, :, t * TT : (t + 1) * TT], in_=rt)

    _legalize_waits(nc)
    return nc


def _get_program(TPC):
    if TPC not in _CACHE:
        _CACHE[TPC] = _build_device_program(TPC)
    return _CACHE[TPC]


_LAST_IN_MAPS = None


def _device_run(nc, in_maps, **kw):
    return run_bass_kernel_spmd(nc, in_maps, core_ids=list(range(NCORES)), **kw)


# ---------------------------------------------------------------------------
def kernel(inputs, lf1_cache, lf2_cache, w1, b1, w2, b2, gamma,
           seq_start_loc, seq_lens, max_len):
    global _LAST_IN_MAPS
    inputs = np.asarray(inputs, np.float32)
    lf1_cache = np.asarray(lf1_cache, np.float32)
    lf2_cache = np.asarray(lf2_cache, np.float32)
    w1 = np.asarray(w1, np.float32)
    b1 = np.asarray(b1, np.float32)
    w2 = np.asarray(w2, np.float32)
    b2 = np.asarray(b2, np.float32)
    gamma = np.asarray(gamma, np.float32)
    seq_start_loc = np.asarray(seq_start_loc, np.int64)
    seq_lens = np.asarray(seq_lens, np.int64)
    max_len = int(np.asarray(max_len))

    T, E_ = inputs.shape
    B = seq_lens.shape[0]
    assert E_ == E and T % NCORES == 0
    TPC = T // NCORES
    assert TPC % TT == 0

    nc = _get_program(TPC)

    # host prep
    b2eff = b2 + (w2[:, :, 0] + w2[:, :, 1]) @ b1            # [E]
    w1t = _rne11(np.transpose(w1, (1, 2, 0)))                # [E, 2, H]
    w2t = _rne11(np.transpose(w2, (1, 2, 0)))                # [H, 2, E]
    XE = np.concatenate(
        [np.zeros((2, E), np.float32), inputs, np.zeros((2, E), np.float32)],
        axis=0,
    )

    in_maps = []
    for c in range(NCORES):
        xe_c = XE[c * TPC : c * TPC + TPC + 4]               # [TPC+4, E]
        xt = _rne11(xe_c.T)                                  # [E, TPC+4]
        xres = np.ascontiguousarray(inputs[c * TPC : (c + 1) * TPC].T) \
            + b2eff[:, None]                                 # [E, TPC]
        in_maps.append({
            "xt": xt,
            "w1t": w1t,
            "w2t": w2t,
            "xres": np.ascontiguousarray(xres, np.float32),
            "gam": gamma,
        })
    _LAST_IN_MAPS = in_maps

    res = _device_run(nc, in_maps)
    y = np.concatenate(
        [np.ascontiguousarray(res.results[c]["yt"]).T for c in range(NCORES)],
        axis=0,
    )  # [T, E]

    # ---- host fixups: rows s0, s0+1 of each sequence (exact) ----
    W10, W11 = w1[:, :, 0], w1[:, :, 1]
    W20, W21 = w2[:, :, 0], w2[:, :, 1]

    def _norm(out_row):
        var = np.mean(out_row.astype(np.float32) ** 2)
        return gamma * (out_row * (1.0 / np.sqrt(var + EPS)))

    for i in range(B):
        L = int(seq_lens[i])
        s0 = int(seq_start_loc[i])
        if L <= 0:
            continue
        p0 = max_len - L
        x0 = inputs[s0]
        if p0 == 0:
            xm1 = lf1_cache[i, :, 0, 0]
            o1_m1 = lf2_cache[i, :, 0, 0]
        elif p0 == 1:
            xm1 = np.zeros(E, np.float32)
            o1_m1 = W10 @ lf1_cache[i, :, 0, 0] + b1
        else:
            xm1 = np.zeros(E, np.float32)
            o1_m1 = b1
        o1_0 = W10 @ xm1 + W11 @ x0 + b1
        o2_0 = W20 @ o1_m1 + W21 @ o1_0 + b2
        y[s0] = _norm(o2_0 + x0)
        if L >= 2:
            x1 = inputs[s0 + 1]
            o1_1 = W10 @ x0 + W11 @ x1 + b1
            o2_1 = W20 @ o1_0 + W21 @ o1_1 + b2
            y[s0 + 1] = _norm(o2_1 + x1)

    # ---- caches ----
    lf1 = np.zeros((B, E, 1, 1), np.float32)
    lf2 = np.zeros((B, H, 1, 1), np.float32)
    for i in range(B):
        L = int(seq_lens[i])
        s0 = int(seq_start_loc[i])
        if L <= 0:
            continue
        p0 = max_len - L
        xlast = inputs[s0 + L - 1]
        if L >= 2:
            xprev = inputs[s0 + L - 2]
        elif p0 == 0:
            xprev = lf1_cache[i, :, 0, 0]
        else:
            xprev = np.zeros(E, np.float32)
        lf1[i, :, 0, 0] = xlast
        lf2[i, :, 0, 0] = W10 @ xprev + W11 @ xlast + b1

    return y, lf1, lf2


# revision 11
# speedup vs baseline: 1.0710x; 1.0438x over previous
"""Trainium2 Bass kernel: LocalizedFiltering (2x causal conv k=2 + residual + RMSNorm)
over ragged packed sequences.

Strategy:
  - Split the 24576 packed tokens evenly across 8 NeuronCores (3072 tokens/core)
    with a 2-token halo; conv weights replicated.
  - Device computes conv1 -> (DRAM spill) -> conv2 + residual + RMSNorm on
    channels-on-partitions transposed layout, in float32r (full-rate PE matmul,
    RNE-11-bit mantissa) with fp32 PSUM accumulation.
  - Sequence-boundary rows (2 per sequence start) and the conv caches are
    recomputed exactly on host (a handful of matvecs).
"""
import numpy as np

import concourse.bass as bass
import concourse.tile as tile
from concourse import mybir
from concourse.bass_utils import run_bass_kernel_spmd
import bass_rust

EPS = 1e-6
P = 128
NCORES = 8
E = 2048
H = 1024
KE = E // P   # 16
KH = H // P   # 8
TT = 256      # tokens per device tile


# ---------------------------------------------------------------------------
# Wait legalization: the walrus build here allows at most one sync wait per
# instruction (two for EventSemaphore); hoist excess waits onto NoOps.
_WAIT_CAPS = {"Matmult": 0, "EventSemaphore": 2}


def _legalize_waits(nc):
    n_split = 0
    for fn in nc.m.functions:
        for bb in fn.blocks:
            insts = list(bb.instructions)
            out = []
            changed = False
            for ins in insts:
                si = ins.sync_info
                waits = list(si.on_wait) if si is not None else []
                cap = _WAIT_CAPS.get(ins.opcode, 1)
                if len(waits) > cap:
                    keep, hoist = waits[:cap], waits[cap:]
                    for w in hoist:
                        n_split += 1
                        nop = mybir.InstNoOp(
                            name=f"WSPLIT-{n_split}", ins=[], outs=[]
                        )
                        nop.engine = ins.engine
                        nop.sync_info = bass_rust.SyncInfo(on_wait=[w], on_update=[])
                        out.append(nop)
                    ins.sync_info = bass_rust.SyncInfo(
                        on_wait=keep, on_update=list(si.on_update)
                    )
                    changed = True
                out.append(ins)
            if changed:
                bb.instructions = out
    return n_split


# ---------------------------------------------------------------------------
# Host-side round-to-nearest-even to 11 mantissa bits (the f32r format the
# PE consumes; verified bit-exact against the DVE's fp32->f32r conversion).
def _rne11(a):
    u = np.ascontiguousarray(a, dtype=np.float32).view(np.uint32)
    drop = 12
    half = np.uint32(1 << (drop - 1))
    even = (~(u >> drop) & np.uint32(1)).astype(np.uint32)
    mask = np.uint32((0xFFFFFFFF >> drop) << drop)
    r = ((u + half - even) & mask).view(np.float32)
    return np.ascontiguousarray(r)


# ---------------------------------------------------------------------------
_CACHE = {}


def _build_device_program(TPC):
    NT = TPC // TT
    TTE = TT + 3   # X tile columns (2 halo + 1 even-pad for f32r)
    TO = TT + 2    # o1e columns per tile (even, required by f32r matmul)
    f32r = mybir.dt.float32r
    f32 = mybir.dt.float32

    nc = bass.Bass(target_bir_lowering=False)
    dxt = nc.dram_tensor("xt", [E, TPC + 4], f32r, kind="ExternalInput").ap()
    dw1 = nc.dram_tensor("w1t", [KH, E, 2, P], f32r, kind="ExternalInput").ap()
    dw2 = nc.dram_tensor("w2t", [KE, H, 2, P], f32r, kind="ExternalInput").ap()
    dxr = nc.dram_tensor("xres", [E, TPC], f32, kind="ExternalInput").ap()
    dyt = nc.dram_tensor("yt", [E, TPC], f32, kind="ExternalOutput").ap()

    xtr = dxt.rearrange("(k p) n -> p k n", p=P)      # [128, 16, TPC+2]
    xrr = dxr.rearrange("(k p) n -> p k n", p=P)      # [128, 16, TPC]
    ytr = dyt.rearrange("(k p) n -> p k n", p=P)      # [128, 16, TPC]

    with tile.TileContext(nc) as tc:
        with (
            tc.tile_pool(name="wpool", bufs=1) as wpool,
            tc.tile_pool(name="const", bufs=1) as constp,
            tc.tile_pool(name="act", bufs=2) as actp,
            tc.tile_pool(name="o1", bufs=2) as o1p,
            tc.tile_pool(name="sq", bufs=3) as sqp,
            tc.tile_pool(name="small", bufs=4) as smallp,
            tc.tile_pool(name="rb", bufs=2) as rbp,
            tc.tile_pool(name="mm", bufs=6, space="PSUM") as mmp,
            tc.tile_pool(name="vps", bufs=2, space="PSUM") as vpsp,
            tc.tile_pool(name="dram", bufs=2, space="DRAM") as dramp,
        ):
            ones_f = constp.tile([P, 1], f32)
            nc.vector.memset(ones_f, 1.0)
            ones_r = constp.tile([P, 1], f32r)
            nc.vector.tensor_copy(out=ones_r, in_=ones_f)
            epst = constp.tile([1, 1], f32)
            nc.vector.memset(epst, EPS)
            o1d = dramp.tile([NT, H, TO], f32r)

            # ---- pass A: conv1 (no bias; folded into xres) ----
            tw1 = wpool.tile([P, KH, KE, 2, P], f32r, tag="w")
            for co in range(KH):
                nc.sync.dma_start(
                    out=tw1[:, co],
                    in_=dw1[co].rearrange("(k p) t m -> p k t m", p=P),
                )
            for t in range(NT):
                xa = actp.tile([P, KE, TTE], f32r, tag="act")
                nc.scalar.dma_start(out=xa, in_=xtr[:, :, t * TT : t * TT + TTE])
                o1sb = o1p.tile([P, KH, TO], f32r, tag="o1")
                for co in range(KH):
                    ps = mmp.tile([P, TO], f32, tag="mm")
                    n_mm = 0
                    for k in range(KE):
                        for tap in range(2):
                            nc.tensor.matmul(
                                ps,
                                tw1[:, co, k, tap, :],
                                xa[:, k, tap : tap + TO],
                                start=(n_mm == 0),
                                stop=(n_mm == 2 * KE - 1),
                            )
                            n_mm += 1
                    nc.vector.tensor_copy(out=o1sb[:, co, :], in_=ps)
                nc.gpsimd.dma_start(
                    out=o1d[t].rearrange("(k p) n -> p k n", p=P), in_=o1sb
                )

            # ---- pass B: conv2 + residual (+b2eff via xres) + RMSNorm ----
            tw2 = wpool.tile([P, KE, KH, 2, P], f32r, tag="w")
            for co in range(KE):
                nc.sync.dma_start(
                    out=tw2[:, co],
                    in_=dw2[co].rearrange("(k p) t m -> p k t m", p=P),
                )
            for t in range(NT):
                o1t = o1p.tile([P, KH, TO], f32r, tag="o1")
                nc.scalar.dma_start(
                    out=o1t, in_=o1d[t].rearrange("(k p) n -> p k n", p=P)
                )
                rt = actp.tile([P, KE, TT], f32, tag="act")
                nc.scalar.dma_start(out=rt, in_=xrr[:, :, t * TT : (t + 1) * TT])
                vps = vpsp.tile([1, TT], f32, tag="v")
                sq_tiles = [None] * KE
                for co in range(KE):
                    ps2 = mmp.tile([P, TT], f32, tag="mm")
                    n_mm = 0
                    for k in range(KH):
                        for tap in range(2):
                            nc.tensor.matmul(
                                ps2,
                                tw2[:, co, k, tap, :],
                                o1t[:, k, tap : tap + TT],
                                start=(n_mm == 0),
                                stop=(n_mm == 2 * KH - 1),
                            )
                            n_mm += 1
                    nc.vector.tensor_add(
                        out=rt[:, co, :], in0=rt[:, co, :], in1=ps2
                    )
                    sq = sqp.tile([P, TT], f32r, tag="sq")
                    nc.scalar.activation(
                        out=sq, in_=rt[:, co, :],
                        func=mybir.ActivationFunctionType.Square,
                    )
                    sq_tiles[co] = sq
                    # issue the variance matmul one chunk late: its DVE input
                    # is then already materialized when PE reaches it
                    if co >= 1:
                        nc.tensor.matmul(
                            vps, ones_r, sq_tiles[co - 1],
                            start=(co == 1), stop=False,
                            skip_group_check=True,
                        )
                nc.tensor.matmul(
                    vps, ones_r, sq_tiles[KE - 1],
                    start=False, stop=True, skip_group_check=True,
                )
                vsb = smallp.tile([1, TT], f32)
                nc.scalar.activation(
                    out=vsb,
                    in_=vps,
                    func=mybir.ActivationFunctionType.Sqrt,
                    bias=epst,
                    scale=1.0 / E,
                )
                rec = smallp.tile([1, TT], f32)
                nc.vector.reciprocal(out=rec, in_=vsb)
                dvrec = dramp.tile([1, TT], f32, tag="vrec")
                nc.gpsimd.dma_start(out=dvrec, in_=rec)
                rb = rbp.tile([P, TT], f32)
                nc.gpsimd.dma_start(out=rb, in_=dvrec.to_broadcast([P, TT]))
                for co in range(KE):
                    nc.vector.tensor_mul(
                        out=rt[:, co, :], in0=rt[:, co, :], in1=rb
                    )
                nc.gpsimd.dma_start(out=ytr[:, :, t * TT : (t + 1) * TT], in_=rt)

    _legalize_waits(nc)
    return nc


def _get_program(TPC):
    if TPC not in _CACHE:
        _CACHE[TPC] = _build_device_program(TPC)
    return _CACHE[TPC]


_LAST_IN_MAPS = None


def _device_run(nc, in_maps, **kw):
    return run_bass_kernel_spmd(nc, in_maps, core_ids=list(range(NCORES)), **kw)


# ---------------------------------------------------------------------------
def kernel(inputs, lf1_cache, lf2_cache, w1, b1, w2, b2, gamma,
           seq_start_loc, seq_lens, max_len):
    global _LAST_IN_MAPS
    inputs = np.asarray(inputs, np.float32)
    lf1_cache = np.asarray(lf1_cache, np.float32)
    lf2_cache = np.asarray(lf2_cache, np.float32)
    w1 = np.asarray(w1, np.float32)
    b1 = np.asarray(b1, np.float32)
    w2 = np.asarray(w2, np.float32)
    b2 = np.asarray(b2, np.float32)
    gamma = np.asarray(gamma, np.float32)
    seq_start_loc = np.asarray(seq_start_loc, np.int64)
    seq_lens = np.asarray(seq_lens, np.int64)
    max_len = int(np.asarray(max_len))

    T, E_ = inputs.shape
    B = seq_lens.shape[0]
    assert E_ == E and T % NCORES == 0
    TPC = T // NCORES
    assert TPC % TT == 0

    nc = _get_program(TPC)

    # host prep
    b2eff = b2 + (w2[:, :, 0] + w2[:, :, 1]) @ b1            # [E]
    w1t = np.transpose(w1, (1, 2, 0))                        # [E, 2, H]
    w2t = np.transpose(w2, (1, 2, 0))                        # [H, 2, E]
    w1s = _rne11(np.stack(
        [w1t[:, :, co * P : (co + 1) * P] for co in range(KH)]))  # [KH,E,2,P]
    w2s = _rne11(np.stack(
        [w2t[:, :, co * P : (co + 1) * P] for co in range(KE)]))  # [KE,H,2,P]
    XE = np.concatenate(
        [np.zeros((2, E), np.float32), inputs, np.zeros((2, E), np.float32)],
        axis=0,
    )

    in_maps = []
    for c in range(NCORES):
        xe_c = XE[c * TPC : c * TPC + TPC + 4]               # [TPC+4, E]
        xt = _rne11(xe_c.T)                                  # [E, TPC+4]
        xres = np.ascontiguousarray(inputs[c * TPC : (c + 1) * TPC].T) \
            + b2eff[:, None]                                 # [E, TPC]
        in_maps.append({
            "xt": xt,
            "w1t": w1s,
            "w2t": w2s,
            "xres": np.ascontiguousarray(xres, np.float32),
        })
    _LAST_IN_MAPS = in_maps

    res = _device_run(nc, in_maps)
    y = np.concatenate(
        [np.ascontiguousarray(res.results[c]["yt"]).T for c in range(NCORES)],
        axis=0,
    )  # [T, E]
    if not np.all(gamma == 1.0):
        y *= gamma[None, :]

    # ---- host fixups: rows s0, s0+1 of each sequence (exact) ----
    W10, W11 = w1[:, :, 0], w1[:, :, 1]
    W20, W21 = w2[:, :, 0], w2[:, :, 1]

    def _norm(out_row):
        var = np.mean(out_row.astype(np.float32) ** 2)
        return gamma * (out_row * (1.0 / np.sqrt(var + EPS)))

    for i in range(B):
        L = int(seq_lens[i])
        s0 = int(seq_start_loc[i])
        if L <= 0:
            continue
        p0 = max_len - L
        x0 = inputs[s0]
        if p0 == 0:
            xm1 = lf1_cache[i, :, 0, 0]
            o1_m1 = lf2_cache[i, :, 0, 0]
        elif p0 == 1:
            xm1 = np.zeros(E, np.float32)
            o1_m1 = W10 @ lf1_cache[i, :, 0, 0] + b1
        else:
            xm1 = np.zeros(E, np.float32)
            o1_m1 = b1
        o1_0 = W10 @ xm1 + W11 @ x0 + b1
        o2_0 = W20 @ o1_m1 + W21 @ o1_0 + b2
        y[s0] = _norm(o2_0 + x0)
        if L >= 2:
            x1 = inputs[s0 + 1]
            o1_1 = W10 @ x0 + W11 @ x1 + b1
            o2_1 = W20 @ o1_0 + W21 @ o1_1 + b2
            y[s0 + 1] = _norm(o2_1 + x1)

    # ---- caches ----
    lf1 = np.zeros((B, E, 1, 1), np.float32)
    lf2 = np.zeros((B, H, 1, 1), np.float32)
    for i in range(B):
        L = int(seq_lens[i])
        s0 = int(seq_start_loc[i])
        if L <= 0:
            continue
        p0 = max_len - L
        xlast = inputs[s0 + L - 1]
        if L >= 2:
            xprev = inputs[s0 + L - 2]
        elif p0 == 0:
            xprev = lf1_cache[i, :, 0, 0]
        else:
            xprev = np.zeros(E, np.float32)
        lf1[i, :, 0, 0] = xlast
        lf2[i, :, 0, 0] = W10 @ xprev + W11 @ xlast + b1

    return y, lf1, lf2
